# revision 50
# baseline (speedup 1.0000x reference)
"""Binarized 3x3 conv (N=32, C=256->256, H=W=56, pad 1) on 8 TRN2 NeuronCores.

Sharding: data-parallel over batch (4 images per core), weights replicated.

Math: binarize exactly via
  xb = (x >= 0) - 0.5            in {+-0.5}  (exact in fp8 e4m3)
  wb = (w >= 0) - 0.5            in {+-0.5}  (exact in fp8 e4m3)
so every product is exactly +-0.25 and fp32 PSUM accumulation is exact
(quarter-integer partial sums, |.| <= 576 << 2^22). The output drain applies
scale=4.0 to restore the +-1-product conv result. sign(0)=+1 is honored.

Conv as matmul: each binarized image lives flat in SBUF on a padded 58-row
grid with a 64B row pitch, so for each kernel tap (kh,kw) the needed input
window is a CONTIGUOUS span shifted by (kh-1)*64+(kw-1). THREE byte-shifted
copies of each grid (base offsets DELTA[kw] = 1/0/15) make every tap's span
base 16B-aligned: an aligned fp8 DoubleRow rhs streams ~1.5x faster than a
misaligned one (HW-measured). Outputs are computed on the padded grid
(448-wide spans = 7 padded rows) and garbage columns dropped at drain time.

TensorE: fp8 DoubleRow matmuls contract all 256 input channels in one
instruction (K=128 partitions x 2 interleaved weights/cell), 9 accumulating
matmuls (one per tap) per output tile. 2 co-chunks x 4 images x 8 row-groups
x 9 taps = 576 matmuls per core.

Engine placement (each was measured, not guessed): binarize of all three
copies on DVE only — GPSIMD tensor_scalar is several times slower and
matmul-gating copies on it serialized the pipeline to ~600us/rep; conv
drains all on ACT; weight transpose-drains on DVE; x loads split across the
two HWDGE rings (one per ci-chunk) and outputs alternate the same rings.
Emission interleaves each image's binarize with the previous image's conv
groups so the DVE FIFO never parks work behind a not-yet-loaded image.
Border memsets run once (rep 0): binarize rewrites only interiors.

Weights: ONE contiguous DMA loads w[o, i, kh, kw] as [o_local=128 part,
(oc, i, tap)] (256 descriptors of 9216B — the HBM-contiguous axis (i, tap)
lands on the SBUF free axis). The o<->i transpose needed for the matmul
lhsT layout [ci_local][two][co] is done on-chip: 36 PE transpose-mode
matmuls of 128x128 f32 blocks (strided columns, stride 9) into PSUM, each
drained by a DVE tensor_scalar that fuses the binarize to {+-0.5} fp8 and
scatters into the DoubleRow layout [tap][two][co].

Pitfalls that cost real debugging time: (1) SBUF is 224KiB/partition
physical but only ~208KiB usable — bass does NOT assert on overrun; a
237KB high-water silently corrupted runtime state and wedged the device
(NRT_EXEC_UNIT_UNRECOVERABLE). _get_nc() now guards this. (2) A SWDGE
f32->bf16 cast-DMA for x runs at ~33GB/s (~390us for 12.8MB) — avoid.
"""

import os
os.environ.setdefault("CONCOURSE_SCRUB_NEFF_DEBUG_INFO", "1")

import numpy as np

import concourse.bass as bass
import concourse.mybir as mybir
import concourse.tile as tile
from concourse import bacc, bass_utils, masks

N_CORES = 8
N, CIN, H, W = 32, 256, 56, 56
COUT, KS = 256, 3
NPC = N // N_CORES          # images per core
HP, WP = H + 2, W + 2       # padded spatial (58 rows)
# ALIGNED=True: 64B row pitch + three byte-shifted copies of each binarized
# grid (indexed by kw, grid base LEAD + DELTA[kw]) so every tap's rhs span
# base (= base + WROW + rg*FREE + (kh-1)*WROW + (kw-1)) is 16B-aligned —
# measured on HW, an aligned DR span streams ~1.5x faster than a misaligned
# one. ALIGNED=False: single copy at 58B pitch (some spans misaligned).
ALIGNED = True
if ALIGNED:
    WROW = 64               # padded row pitch (%16==0 keeps row shifts aligned)
    NCOPY = 3
    DELTA = (1, 0, 15)
    LEAD = 16               # min span base = LEAD+DELTA[0]+WROW-WROW-1 >= 0
    NROW_GROUPS = 8         # 8 groups x 7 rows: FREE=448. Do NOT use 7x512:
                            # a DR matmul with rhs pair-total 1024 (the fp8
                            # moving-operand ceiling) wedges the device
                            # (NRT_EXEC_UNIT_UNRECOVERABLE, reproduced twice)
else:
    WROW = W + 2            # 58
    NCOPY = 1
    DELTA = (0, 0, 0)
    LEAD = 64
    NROW_GROUPS = 7         # 7 groups x 8 rows (FREE=464)
GRID = HP * WROW
CHUNK = LEAD + GRID + 16 + (16 - (LEAD + GRID) % 16) % 16  # %16 == 0
ROWS_PER_GROUP = H // NROW_GROUPS
FREE = ROWS_PER_GROUP * WROW        # <= 512 (one PSUM bank, fp32)
CI_CHUNKS = CIN // 128
CO_CHUNKS = COUT // 128

F32 = mybir.dt.float32
BF16 = mybir.dt.bfloat16
FP8 = mybir.dt.float8e4
ALU = mybir.AluOpType
AF = mybir.ActivationFunctionType
DR = mybir.MatmulPerfMode.DoubleRow

# tap groups for the weight-transpose drains: 4+4+1 blocks per 512-f32 PSUM bank
TAP_GROUPS = [(0, 4), (4, 4), (8, 1)]


def _body(tc, x_d, w_d, b_d, o_d, repeats=1, parts="full"):
    nc = tc.nc

    from contextlib import ExitStack
    ctx = ExitStack()
    with ctx:
        const_pool = ctx.enter_context(tc.tile_pool(name="const", bufs=1))
        # bufs=2: rep r+1's transpose-drains write the other wd8 buffer, so
        # they never WAR-wait on rep r's last matmuls still reading wd8
        wd_pool = ctx.enter_context(tc.tile_pool(name="wd", bufs=2))
        wsb_pool = ctx.enter_context(tc.tile_pool(name="wsb", bufs=1))
        xpad_pool = ctx.enter_context(tc.tile_pool(name="xpad", bufs=1))
        xin_pool = ctx.enter_context(tc.tile_pool(name="xin", bufs=2))
        # ob bufs=1: group g+1's first drain WAR-waits on group g's out DMA,
        # but that DMA (~4.5us) completes well inside g+1's ~10us MM phase
        out_pool = ctx.enter_context(tc.tile_pool(name="outs", bufs=1))

        ident = const_pool.tile([128, 128], F32, tag="ident", name="ident")
        masks.make_identity(nc, ident[:])

        bias_sb = const_pool.tile([128, CO_CHUNKS], F32, tag="bias",
                                  name="bias_sb")

        o_d3 = [[o_d[n, cc * 128:(cc + 1) * 128].rearrange("c h w -> c (h w)")
                 for cc in range(CO_CHUNKS)] for n in range(NPC)]

        for rep in range(repeats):
            # ---- weight phase: contiguous DMAs + on-chip transpose ----
            # wsb[cc]: [o_local=128, (i, tap)] — HBM-contiguous (i, tap) on
            # the free axis, so this is 128 descriptors of 9216B per chunk.
            # One tile per co-chunk (so cc0's transposes depend only on cc0's
            # DMA), issued on the ACT HWDGE ring (nc.scalar) while the SP
            # ring streams x.
            w_src = w_d.rearrange("(oc p) i kh kw -> p oc (i kh kw)", p=128)
            wsb = []
            for cc in range(CO_CHUNKS):
                wt = wsb_pool.tile([128, CIN * KS * KS], F32,
                                   tag=f"wsb{cc}", name=f"wsb{rep}_{cc}")
                # halves by ci-chunk: the two=0 transposes only wait for the
                # first half, shortening the weight-phase critical chain
                half = 128 * KS * KS
                nc.scalar.dma_start(wt[:, :half], w_src[:, cc, :half])
                nc.scalar.dma_start(wt[:, half:], w_src[:, cc, half:])
                wsb.append(wt)
            if rep == 0:
                nc.scalar.dma_start(bias_sb[:],
                                    b_d.rearrange("(c p) -> p c", p=128))
            wviews = [t[:].rearrange("p (i t) -> p i t", t=KS * KS)
                      for t in wsb]

            # wd8[cc]: [128 ci_local, 9*256] fp8, free idx = tap*256 + two*128
            # + co, values (w>=0)-0.5 in {+-0.5}. (lhsT slice per tap:
            # [k][two][m], steps [128, 1] — DoubleRow pairing contracts
            # (k, two) elementwise on both operands.)
            wd8 = []
            for cc in range(CO_CHUNKS):
                wt = wd_pool.tile([128, KS * KS * 256], FP8, tag=f"wd{cc}",
                                  name=f"wd8_{rep}_{cc}")
                wd8.append(wt)

            xpall = xpad_pool.tile([128, NCOPY * NPC * CI_CHUNKS * CHUNK], FP8,
                                   tag="xpall", name=f"xpall{rep}")
            # [part, kw-copy, n, ci-chunk, chunk] — the two ci-chunk grids of
            # one (kw, n) copy are adjacent (stride CHUNK, %16==0) so they
            # form the DoubleRow pair axis of the rhs AP
            xg5 = xpall[:].rearrange("c (k n t s) -> c k n t s",
                                     n=NPC, t=CI_CHUNKS, s=CHUNK)

            def emit_weight_cc(cc, wtpsum):
                wt3 = wd8[cc][:].rearrange("k (t x) -> k t x", t=KS * KS)
                for two in range(CI_CHUNKS):
                    for g, (t0, tn) in enumerate(TAP_GROUPS):
                        pt = wtpsum.tile([128, 512], F32, tag="wtp",
                                         name=f"wtp{rep}_{cc}_{two}_{g}")
                        for j in range(tn):
                            nc.tensor.transpose(
                                pt[:, j * 128:(j + 1) * 128],
                                wviews[cc][:, two * 128:(two + 1) * 128,
                                           t0 + j],
                                ident[:])
                        # drain + binarize: {+-0.5} fp8, scattered to
                        # [tap][two][co] (dst strides: tap 256, co 1)
                        nc.vector.tensor_scalar(
                            wt3[:, t0:t0 + tn, two * 128:(two + 1) * 128],
                            pt[:, :tn * 128].rearrange(
                                "k (t x) -> k t x", x=128),
                            0.0, 0.5, op0=ALU.is_ge, op1=ALU.subtract)

            def emit_memsets():
                # borders/pads of all 24 padded grids zeroed once (rep 0):
                # binarize only rewrites interiors [1:57, 1:57). Per kw-copy
                # (grids at byte base LEAD+DELTA[kw]): front+row0, row57+tail,
                # left col 0, right cols 57..63. Taps never read below
                # base+WROW-1 or past base+GRID+1, both inside these bands.
                xg = xpall[:].rearrange("c (k g s) -> c k g s",
                                        k=NCOPY, s=CHUNK)
                for kw in range(NCOPY):
                    g0 = LEAD + DELTA[kw]
                    nc.gpsimd.memset(xg[:, kw, :, 0:g0 + WROW], 0.0)
                    nc.gpsimd.memset(
                        xg[:, kw, :, g0 + (HP - 1) * WROW:CHUNK], 0.0)
                    xgrid = xg[:, kw, :, g0:g0 + GRID] \
                        .rearrange("c g (h w) -> c g h w", w=WROW)
                    nc.gpsimd.memset(xgrid[:, :, 1:HP - 1, 0:1], 0.0)
                    nc.gpsimd.memset(xgrid[:, :, 1:HP - 1, WP - 1:WROW], 0.0)

            xraw_tiles = {}

            def emit_xdma(n):
                # one full-height DMA per (n, ci-chunk): 128 descriptors of
                # 12.5KB, alternating the two HWDGE rings. (A SWDGE
                # f32->bf16 cast-DMA was tried to halve staging: the cast
                # path runs at ~33GB/s and took ~390us — never again.)
                xraws = [xin_pool.tile([128, H * W], F32, tag="xraw",
                                       name=f"xraw{rep}_{n}_{two}")
                         for two in range(CI_CHUNKS)]
                xraw_tiles[n] = xraws
                for two in range(CI_CHUNKS):
                    eng = nc.sync if two == 0 else nc.scalar
                    eng.dma_start(
                        xraws[two][:],
                        x_d[n, two * 128:(two + 1) * 128].rearrange(
                            "c h w -> c (h w)"))

            def emit_binarize(n):
                # Writes the interior of all kw-copies. ALL on DVE: GPSIMD
                # tensor_scalar is several times slower and having matmul-
                # gating copies on it serialized the whole pipeline behind
                # GPSIMD (measured ~600us/rep). Conv drains live on ACT so
                # DVE's queue is binarize + weight drains only.
                for two in range(CI_CHUNKS):
                    xr_in = xraw_tiles[n][two][:].rearrange(
                        "c (h w) -> c h w", w=W)
                    for kw in range(NCOPY):
                        xg_in = xg5[:, kw, n, two,
                                    LEAD + DELTA[kw]:LEAD + DELTA[kw] + GRID
                                    ].rearrange("c (h w) -> c h w", w=WROW)
                        nc.vector.tensor_scalar(
                            xg_in[:, 1:H + 1, 1:W + 1],
                            xr_in[:],
                            0.0, 0.5, op0=ALU.is_ge, op1=ALU.subtract)

            # HAM warmup: the PE is idle until the first weight DMA lands
            # (~4us) and would then run its first ~3.4us of real work at
            # 1.2 GHz (cold clock-gate). Dummy identity matmuls during the
            # DMA wait release the throttle before the transposes start.
            # (Transpose-mode matmuls don't count as PE-busy for HAM.)
            if rep == 0:
                with tc.tile_pool(name="warm", bufs=1, space="PSUM") as wp:
                    warm = wp.tile([128, 128], F32, tag="warm", name="warm")
                    for i in range(8):
                        nc.tensor.matmul(warm[:], ident[:], ident[:],
                                         start=True, stop=True)

            # emission order = scheduling priority: cc0 weights, then the
            # x DMAs (SP ring starts draining immediately), image-0
            # binarize on GPSIMD, cc1 weights; images 1..3 binarize is
            # emitted interleaved with the conv groups below so the DVE
            # FIFO never parks a conv drain behind a late image's binarize
            n_inputs = NPC if parts not in ("mmonly", "mmraw", "mmraweven", "mmrawsame") else 0
            with tc.tile_pool(name="wtp", bufs=2, space="PSUM") as wtpsum:
                emit_weight_cc(0, wtpsum)
                if rep == 0 or n_inputs == 0:
                    # borders are static zeros: binarize only rewrites the
                    # interiors, so later reps reuse rep 0's borders (input-
                    # less ablations re-memset so xpall has a writer per rep)
                    emit_memsets()
                for n in range(n_inputs):
                    emit_xdma(n)
                if n_inputs:
                    emit_binarize(0)
                emit_weight_cc(1, wtpsum)
                if n_inputs > 1:
                    emit_binarize(1)
            # per image view [k, kw, two, s]: the two-axis (stride CHUNK,
            # %16==0) is the DoubleRow pair axis of the rhs
            xp = [xg5[:, :, n] for n in range(NPC)]

            # ---- conv phase ----
            # per-row-group PSUM tiles rotating through all 8 banks: group
            # g+1's first matmul into a bank only waits for a drain from
            # ~1.5 groups earlier, so TensorE never stalls on drains
            with tc.tile_pool(name="cpsum", bufs=8, space="PSUM") as cpsum:
                ngroups = NPC * CO_CHUNKS if parts != "nomm" else 0
                for gi in range(ngroups):
                    n, cc = divmod(gi, CO_CHUNKS)
                    if cc == 0 and 2 <= n + 1 < n_inputs:
                        emit_binarize(n + 1)
                    pps = [cpsum.tile([128, 512], F32, tag="cps",
                                      name=f"cps{rep}_{cc}_{n}_{rg}")
                           for rg in range(NROW_GROUPS)]
                    # rg-outer / tap-inner: consecutive matmuls stream
                    # 98%-overlapping rhs spans (offsets +-1, +-58), hitting
                    # the HW span-reuse fast path (measured: re-streaming an
                    # identical span is ~1.4x faster than a fresh one; LDW
                    # is emitted per-matmul either way, so tap-inner loses
                    # nothing on the weight side)
                    if parts == "wre":
                        # A/B probe: tap-outer / rg-inner — 8 consecutive
                        # matmuls share one lhsT; wins iff the backend
                        # dedups/hides consecutive identical weight loads
                        for ki, kpos in enumerate(range(KS * KS)):
                            kh, kw = divmod(kpos, KS)
                            lhsT = wd8[cc][:, kpos * 256:(kpos + 1) * 256] \
                                .rearrange("k (two m) -> k two m", two=2)
                            cp = kw if NCOPY == KS else 0
                            for rg in range(NROW_GROUPS):
                                off = (LEAD + DELTA[cp] + WROW + rg * FREE
                                       + (kh - 1) * WROW + (kw - 1))
                                rhs = xp[n][:, cp, :, off:off + FREE]
                                nc.tensor.matmul(
                                    pps[rg][:, :FREE], lhsT,
                                    rhs, start=(ki == 0),
                                    stop=(ki == KS * KS - 1),
                                    perf_mode=DR)
                    rgs = () if parts == "wre" else range(NROW_GROUPS)
                    for rg in rgs:
                        # tap order kh-major (kpos): measured equal-or-better
                        # than kw-major copy-grouped order, within noise
                        for ki, kpos in enumerate(range(KS * KS)):
                            kh, kw = divmod(kpos, KS)
                            lhsT = wd8[cc][:, kpos * 256:(kpos + 1) * 256] \
                                .rearrange("k (two m) -> k two m", two=2)
                            # span base inside the kw-copy: 16B-aligned by
                            # construction (LEAD+DELTA[kw]+(kw-1) == 64,
                            # and WROW, FREE are multiples of 16)
                            cp = kw if NCOPY == KS else 0
                            off = (LEAD + DELTA[cp] + WROW + rg * FREE
                                   + (kh - 1) * WROW + (kw - 1))
                            rhs = xp[n][:, cp, :, off:off + FREE]
                            nc.tensor.matmul(
                                pps[rg][:, :FREE], lhsT,
                                rhs, start=(ki == 0),
                                stop=(ki == KS * KS - 1),
                                perf_mode=DR)
                    ob = out_pool.tile([128, NROW_GROUPS * ROWS_PER_GROUP * W],
                                       F32, tag="ob",
                                       name=f"ob{rep}_{cc}_{n}")
                    # per-row-group drains (x4 restores the +-0.25 products),
                    # alternating ACT/DVE so the serial drain tail halves
                    # (mmraw/nodrain ablations: 8-col token drains that still
                    # read every PSUM bank so DCE keeps the matmuls)
                    ncol = W if parts not in ("mmraw", "mmraweven", "mmrawsame", "nodrain", "wreuseraw") else 8
                    for rg in range(NROW_GROUPS):
                        drain_in = pps[rg][:, :FREE] \
                            .rearrange("m (r c) -> m r c", c=WROW
                                       )[:, :, 1:ncol + 1]
                        drain_out = ob[:].rearrange(
                            "m (g r c) -> m g r c", g=NROW_GROUPS, c=W
                            )[:, rg, :, :ncol]
                        # all drains on ACT: DVE is fully booked producing
                        # the binarized copies
                        nc.scalar.activation(
                            drain_out, drain_in,
                            AF.Identity, bias=bias_sb[:, cc:cc + 1],
                            scale=4.0)
                    # x rides SWDGE now, so outputs take the two idle HWDGE
                    # rings (alternating SP/ACT); the last group is split so
                    # its early quarters overlap the final drains
                    ob_g = ob[:].rearrange("m (g s) -> m g s", g=NROW_GROUPS)
                    od_g = o_d3[n][cc].rearrange("c (g s) -> c g s",
                                                 g=NROW_GROUPS)
                    if parts in ("noout", "mmraw", "mmraweven", "mmrawsame", "nodrain", "wreuseraw"):
                        # tiny consumer keeps drains/MMs live through DCE
                        nc.gpsimd.dma_start(od_g[:, 0, :64], ob_g[:, 0, :64])
                    elif gi == ngroups - 1:
                        # both HWDGE rings are idle by now; alternate the
                        # quarters so the tail transfer time halves
                        for qi, (lo, hi) in enumerate(
                                ((0, 2), (2, 4), (4, 6), (6, NROW_GROUPS))):
                            eng = nc.sync if qi % 2 == 0 else nc.scalar
                            eng.dma_start(od_g[:, lo:hi], ob_g[:, lo:hi])
                    else:
                        eng = nc.sync if gi % 2 == 0 else nc.scalar
                        eng.dma_start(o_d3[n][cc], ob[:])


_nc_cache = {}


def _get_nc(repeats=1, parts="full"):
    key = (repeats, parts)
    if key not in _nc_cache:
        nc = bacc.Bacc("TRN2", debug=False)
        x_d = nc.dram_tensor("x", [NPC, CIN, H, W], F32, kind="ExternalInput").ap()
        w_d = nc.dram_tensor("w", [COUT, CIN, KS, KS], F32,
                             kind="ExternalInput").ap()
        b_d = nc.dram_tensor("b", [COUT], F32, kind="ExternalInput").ap()
        o_d = nc.dram_tensor("out", [NPC, COUT, H, W], F32,
                             kind="ExternalOutput").ap()
        with tile.TileContext(nc) as tc:
            _body(tc, x_d, w_d, b_d, o_d, repeats=repeats, parts=parts)
        nc.compile()
        # Guard: bass does NOT reliably assert SBUF capacity — an overrun
        # compiles fine, then wedges the device (observed: 237KB high-water
        # silently corrupted runtime state, NRT_EXEC_UNIT_UNRECOVERABLE).
        hi = 0
        for alloc in nc.m.functions[0].allocations:
            if isinstance(alloc, mybir.MemoryLocationSet) \
                    and alloc.kind == "Internal":
                for ml in alloc.memorylocations:
                    d = list(ml.dims)
                    if len(d) == 2 and d[0] == 128:
                        isz = np.dtype(mybir.dt.np(alloc.dtype)).itemsize \
                            if alloc.dtype else 1
                        hi = max(hi, ml.addr + d[1] * isz)
        assert hi <= 224 * 1024, f"SBUF high-water {hi} exceeds 224KiB"
        _nc_cache[key] = nc
    return _nc_cache[key]


# ---- persistent PJRT runner ---------------------------------------------
# bass_utils.run_bass_kernel_spmd builds a FRESH jax.jit closure per call, so
# every invocation re-lowers the module, re-ships the NEFF through axon, and
# re-loads it onto all 8 devices — seconds of overhead per call that has
# nothing to do with device execution. Here the jitted executable (and hence
# the loaded NEFF) is cached: the first call pays compile+load once, later
# calls only dispatch.

_runner_cache = {}


def _get_runner(repeats=1, parts="full"):
    key = (repeats, parts)
    if key in _runner_cache:
        return _runner_cache[key]

    import jax
    from jax.sharding import Mesh, PartitionSpec, NamedSharding
    from jax.experimental.shard_map import shard_map
    from concourse import bass2jax, mybir as mb

    nc = _get_nc(repeats, parts)
    bass2jax.install_neuronx_cc_hook()

    partition_name = (nc.partition_id_tensor.name
                      if nc.partition_id_tensor else None)
    in_names, out_names, out_avals, zero_shapes = [], [], [], []
    for alloc in nc.m.functions[0].allocations:
        if not isinstance(alloc, mb.MemoryLocationSet):
            continue
        name = alloc.memorylocations[0].name
        if alloc.kind == "ExternalInput":
            if name != partition_name:
                in_names.append(name)
        elif alloc.kind == "ExternalOutput":
            out_names.append(name)
            shape = tuple(alloc.tensor_shape)
            dtype = mb.dt.np(alloc.dtype)
            out_avals.append(jax.core.ShapedArray(shape, dtype))
            zero_shapes.append((shape, dtype))
    n_params = len(in_names)
    n_outs = len(out_names)
    in_names = in_names + out_names
    if partition_name is not None:
        in_names = in_names + [partition_name]

    def _body_fn(*args):
        operands = list(args)
        if partition_name is not None:
            operands.append(bass2jax.partition_id_tensor())
        outs = bass2jax._bass_exec_p.bind(
            *operands,
            out_avals=tuple(out_avals),
            in_names=tuple(in_names),
            out_names=tuple(out_names),
            lowering_input_output_aliases=(),
            sim_require_finite=True,
            sim_require_nnan=True,
            nc=nc,
        )
        return tuple(outs)

    devices = jax.devices()[:N_CORES]
    mesh = Mesh(np.asarray(devices), ("core",))
    sharding = NamedSharding(mesh, PartitionSpec("core"))
    donate = tuple(range(n_params, n_params + n_outs))
    fn = jax.jit(
        shard_map(
            _body_fn, mesh=mesh,
            in_specs=(PartitionSpec("core"),) * (n_params + n_outs),
            out_specs=(PartitionSpec("core"),) * n_outs,
            check_rep=False,
        ),
        donate_argnums=donate, keep_unused=True,
    )
    # On-device sharded zero buffers for the donated outputs (regenerated per
    # call — donation consumes them). No host->device traffic involved.
    import jax.numpy as jnp
    zeros_fn = jax.jit(
        lambda: tuple(jnp.zeros((N_CORES * s[0],) + s[1:], d)
                      for s, d in zero_shapes),
        out_shardings=(sharding,) * n_outs,
    )
    runner = {"fn": fn, "zeros_fn": zeros_fn, "sharding": sharding,
              "n_params": n_params, "in_order": in_names[:n_params]}
    _runner_cache[key] = runner
    return runner


def _device_inputs(inputs, runner):
    """Concat per-core shards on axis 0 and put on the 8 devices."""
    import jax
    x, w, b = inputs["x"], inputs["w"], inputs["b"]
    full = {
        "x": np.ascontiguousarray(x, dtype=np.float32),
        "w": np.ascontiguousarray(
            np.broadcast_to(w, (N_CORES,) + tuple(w.shape)).reshape(
                (N_CORES * w.shape[0],) + tuple(w.shape[1:])),
            dtype=np.float32),
        "b": np.ascontiguousarray(
            np.broadcast_to(b, (N_CORES,) + tuple(b.shape)).reshape(
                (N_CORES * b.shape[0],) + tuple(b.shape[1:])),
            dtype=np.float32),
    }
    arrs = [full[name] for name in runner["in_order"]]
    return jax.device_put(arrs, [runner["sharding"]] * len(arrs))


def _exec(runner, dev_in):
    outs = runner["fn"](*dev_in, *runner["zeros_fn"]())
    return outs


def _run(inputs, repeats=1, parts="full", **kwargs):
    x = inputs["x"]
    assert x.shape == (N, CIN, H, W), x.shape
    runner = _get_runner(repeats, parts)
    dev_in = _device_inputs(inputs, runner)
    outs = _exec(runner, dev_in)
    out = np.asarray(outs[0])

    class _Res:
        exec_time_ns = None
        instructions_and_trace = None
    return out, _Res()


def kernel(**inputs) -> np.ndarray:
    out, _ = _run(inputs)
    return out



# revision 51
# speedup vs baseline: 1.2601x; 1.2601x over previous
"""Binarized 3x3 conv (N=32, C=256->256, H=W=56, pad 1) on 8 TRN2 NeuronCores.

Sharding: data-parallel over batch (4 images per core), weights replicated.

Math: binarize exactly via
  xb = (x >= 0) - 0.5            in {+-0.5}  (exact in fp8 e4m3)
  wb = (w >= 0) - 0.5            in {+-0.5}  (exact in fp8 e4m3)
so every product is exactly +-0.25 and fp32 PSUM accumulation is exact
(quarter-integer partial sums, |.| <= 576 << 2^22). The output drain applies
scale=4.0 to restore the +-1-product conv result. sign(0)=+1 is honored.

Conv as matmul: each binarized image lives flat in SBUF on a padded 58-row
grid with a 64B row pitch, so for each kernel tap (kh,kw) the needed input
window is a CONTIGUOUS span shifted by (kh-1)*64+(kw-1). THREE byte-shifted
copies of each grid (base offsets DELTA[kw] = 1/0/15) make every tap's span
base 16B-aligned: an aligned fp8 DoubleRow rhs streams ~1.5x faster than a
misaligned one (HW-measured). Outputs are computed on the padded grid
(448-wide spans = 7 padded rows) and garbage columns dropped at drain time.

TensorE: fp8 DoubleRow matmuls contract all 256 input channels in one
instruction (K=128 partitions x 2 interleaved weights/cell), 9 accumulating
matmuls (one per tap) per output tile. 2 co-chunks x 4 images x 8 row-groups
x 9 taps = 576 matmuls per core.

Engine placement (each was measured, not guessed): binarize of all three
copies on DVE only — GPSIMD tensor_scalar is several times slower and
matmul-gating copies on it serialized the pipeline to ~600us/rep; conv
drains all on ACT; weight transpose-drains on DVE; x loads split across the
two HWDGE rings (one per ci-chunk) and outputs alternate the same rings.
Emission interleaves each image's binarize with the previous image's conv
groups so the DVE FIFO never parks work behind a not-yet-loaded image.
Border memsets run once (rep 0): binarize rewrites only interiors.

Weights: ONE contiguous DMA loads w[o, i, kh, kw] as [o_local=128 part,
(oc, i, tap)] (256 descriptors of 9216B — the HBM-contiguous axis (i, tap)
lands on the SBUF free axis). The o<->i transpose needed for the matmul
lhsT layout [ci_local][two][co] is done on-chip: 36 PE transpose-mode
matmuls of 128x128 f32 blocks (strided columns, stride 9) into PSUM, each
drained by a DVE tensor_scalar that fuses the binarize to {+-0.5} fp8 and
scatters into the DoubleRow layout [tap][two][co].

Pitfalls that cost real debugging time: (1) SBUF is 224KiB/partition
physical but only ~208KiB usable — bass does NOT assert on overrun; a
237KB high-water silently corrupted runtime state and wedged the device
(NRT_EXEC_UNIT_UNRECOVERABLE). _get_nc() now guards this. (2) A SWDGE
f32->bf16 cast-DMA for x runs at ~33GB/s (~390us for 12.8MB) — avoid.
"""

import os
os.environ.setdefault("CONCOURSE_SCRUB_NEFF_DEBUG_INFO", "1")

import numpy as np

import concourse.bass as bass
import concourse.mybir as mybir
import concourse.tile as tile
from concourse import bacc, bass_utils, masks

N_CORES = 8
N, CIN, H, W = 32, 256, 56, 56
COUT, KS = 256, 3
NPC = N // N_CORES          # images per core
HP, WP = H + 2, W + 2       # padded spatial (58 rows)
# ALIGNED=True: 64B row pitch + three byte-shifted copies of each binarized
# grid (indexed by kw, grid base LEAD + DELTA[kw]) so every tap's rhs span
# base (= base + WROW + rg*FREE + (kh-1)*WROW + (kw-1)) is 16B-aligned —
# measured on HW, an aligned DR span streams ~1.5x faster than a misaligned
# one. ALIGNED=False: single copy at 58B pitch (some spans misaligned).
ALIGNED = True
if ALIGNED:
    WROW = 64               # padded row pitch (%16==0 keeps row shifts aligned)
    NCOPY = 3
    DELTA = (1, 0, 15)
    LEAD = 16               # min span base = LEAD+DELTA[0]+WROW-WROW-1 >= 0
    NROW_GROUPS = 8         # 8 groups x 7 rows: FREE=448. Do NOT use 7x512:
                            # a DR matmul with rhs pair-total 1024 (the fp8
                            # moving-operand ceiling) wedges the device
                            # (NRT_EXEC_UNIT_UNRECOVERABLE, reproduced twice)
else:
    WROW = W + 2            # 58
    NCOPY = 1
    DELTA = (0, 0, 0)
    LEAD = 64
    NROW_GROUPS = 7         # 7 groups x 8 rows (FREE=464)
GRID = HP * WROW
CHUNK = LEAD + GRID + 16 + (16 - (LEAD + GRID) % 16) % 16  # %16 == 0
ROWS_PER_GROUP = H // NROW_GROUPS
FREE = ROWS_PER_GROUP * WROW        # <= 512 (one PSUM bank, fp32)
CI_CHUNKS = CIN // 128
CO_CHUNKS = COUT // 128

F32 = mybir.dt.float32
BF16 = mybir.dt.bfloat16
FP8 = mybir.dt.float8e4
ALU = mybir.AluOpType
AF = mybir.ActivationFunctionType
DR = mybir.MatmulPerfMode.DoubleRow

# tap groups for the weight-transpose drains: 4+4+1 blocks per 512-f32 PSUM bank
TAP_GROUPS = [(0, 4), (4, 4), (8, 1)]


def _body(tc, x_d, w_d, b_d, o_d, repeats=1, parts="full"):
    nc = tc.nc

    from contextlib import ExitStack
    ctx = ExitStack()
    with ctx:
        const_pool = ctx.enter_context(tc.tile_pool(name="const", bufs=1))
        # bufs=2: rep r+1's transpose-drains write the other wd8 buffer, so
        # they never WAR-wait on rep r's last matmuls still reading wd8
        wd_pool = ctx.enter_context(tc.tile_pool(name="wd", bufs=2))
        wsb_pool = ctx.enter_context(tc.tile_pool(name="wsb", bufs=1))
        xpad_pool = ctx.enter_context(tc.tile_pool(name="xpad", bufs=1))
        xin_pool = ctx.enter_context(tc.tile_pool(name="xin", bufs=2))
        # ob bufs=1: group g+1's first drain WAR-waits on group g's out DMA,
        # but that DMA (~4.5us) completes well inside g+1's ~10us MM phase
        out_pool = ctx.enter_context(tc.tile_pool(name="outs", bufs=1))

        ident = const_pool.tile([128, 128], F32, tag="ident", name="ident")
        masks.make_identity(nc, ident[:])

        bias_sb = const_pool.tile([128, CO_CHUNKS], F32, tag="bias",
                                  name="bias_sb")

        o_d3 = [[o_d[n, cc * 128:(cc + 1) * 128].rearrange("c h w -> c (h w)")
                 for cc in range(CO_CHUNKS)] for n in range(NPC)]

        for rep in range(repeats):
            # ---- weight phase: contiguous DMAs + on-chip transpose ----
            # wsb[cc]: [o_local=128, (i, tap)] — HBM-contiguous (i, tap) on
            # the free axis, so this is 128 descriptors of 9216B per chunk.
            # One tile per co-chunk (so cc0's transposes depend only on cc0's
            # DMA), issued on the ACT HWDGE ring (nc.scalar) while the SP
            # ring streams x.
            w_src = w_d.rearrange("(oc p) i kh kw -> p oc (i kh kw)", p=128)
            wsb = []
            for cc in range(CO_CHUNKS):
                wt = wsb_pool.tile([128, CIN * KS * KS], F32,
                                   tag=f"wsb{cc}", name=f"wsb{rep}_{cc}")
                # halves by ci-chunk: the two=0 transposes only wait for the
                # first half, shortening the weight-phase critical chain
                half = 128 * KS * KS
                nc.scalar.dma_start(wt[:, :half], w_src[:, cc, :half])
                nc.scalar.dma_start(wt[:, half:], w_src[:, cc, half:])
                wsb.append(wt)
            if rep == 0:
                nc.scalar.dma_start(bias_sb[:],
                                    b_d.rearrange("(c p) -> p c", p=128))
            wviews = [t[:].rearrange("p (i t) -> p i t", t=KS * KS)
                      for t in wsb]

            # wd8[cc]: [128 ci_local, 9*256] fp8, free idx = tap*256 + two*128
            # + co, values (w>=0)-0.5 in {+-0.5}. (lhsT slice per tap:
            # [k][two][m], steps [128, 1] — DoubleRow pairing contracts
            # (k, two) elementwise on both operands.)
            wd8 = []
            for cc in range(CO_CHUNKS):
                wt = wd_pool.tile([128, KS * KS * 256], FP8, tag=f"wd{cc}",
                                  name=f"wd8_{rep}_{cc}")
                wd8.append(wt)

            xpall = xpad_pool.tile([128, NCOPY * NPC * CI_CHUNKS * CHUNK], FP8,
                                   tag="xpall", name=f"xpall{rep}")
            # [part, kw-copy, n, ci-chunk, chunk] — the two ci-chunk grids of
            # one (kw, n) copy are adjacent (stride CHUNK, %16==0) so they
            # form the DoubleRow pair axis of the rhs AP
            xg5 = xpall[:].rearrange("c (k n t s) -> c k n t s",
                                     n=NPC, t=CI_CHUNKS, s=CHUNK)

            def emit_weight_cc(cc, wtpsum):
                wt3 = wd8[cc][:].rearrange("k (t x) -> k t x", t=KS * KS)
                for two in range(CI_CHUNKS):
                    for g, (t0, tn) in enumerate(TAP_GROUPS):
                        pt = wtpsum.tile([128, 512], F32, tag="wtp",
                                         name=f"wtp{rep}_{cc}_{two}_{g}")
                        for j in range(tn):
                            nc.tensor.transpose(
                                pt[:, j * 128:(j + 1) * 128],
                                wviews[cc][:, two * 128:(two + 1) * 128,
                                           t0 + j],
                                ident[:])
                        # drain + binarize: {+-0.5} fp8, scattered to
                        # [tap][two][co] (dst strides: tap 256, co 1)
                        nc.vector.tensor_scalar(
                            wt3[:, t0:t0 + tn, two * 128:(two + 1) * 128],
                            pt[:, :tn * 128].rearrange(
                                "k (t x) -> k t x", x=128),
                            0.0, 0.5, op0=ALU.is_ge, op1=ALU.subtract)

            def emit_memsets():
                # borders/pads of all 24 padded grids zeroed once (rep 0):
                # binarize only rewrites interiors [1:57, 1:57). Per kw-copy
                # (grids at byte base LEAD+DELTA[kw]): front+row0, row57+tail,
                # left col 0, right cols 57..63. Taps never read below
                # base+WROW-1 or past base+GRID+1, both inside these bands.
                xg = xpall[:].rearrange("c (k g s) -> c k g s",
                                        k=NCOPY, s=CHUNK)
                for kw in range(NCOPY):
                    g0 = LEAD + DELTA[kw]
                    nc.gpsimd.memset(xg[:, kw, :, 0:g0 + WROW], 0.0)
                    nc.gpsimd.memset(
                        xg[:, kw, :, g0 + (HP - 1) * WROW:CHUNK], 0.0)
                    xgrid = xg[:, kw, :, g0:g0 + GRID] \
                        .rearrange("c g (h w) -> c g h w", w=WROW)
                    nc.gpsimd.memset(xgrid[:, :, 1:HP - 1, 0:1], 0.0)
                    nc.gpsimd.memset(xgrid[:, :, 1:HP - 1, WP - 1:WROW], 0.0)

            xraw_tiles = {}

            def emit_xdma(n):
                # one full-height DMA per (n, ci-chunk): 128 descriptors of
                # 12.5KB, alternating the two HWDGE rings. (A SWDGE
                # f32->bf16 cast-DMA was tried to halve staging: the cast
                # path runs at ~33GB/s and took ~390us — never again.)
                xraws = [xin_pool.tile([128, H * W], F32, tag="xraw",
                                       name=f"xraw{rep}_{n}_{two}")
                         for two in range(CI_CHUNKS)]
                xraw_tiles[n] = xraws
                for two in range(CI_CHUNKS):
                    eng = nc.sync if two == 0 else nc.scalar
                    eng.dma_start(
                        xraws[two][:],
                        x_d[n, two * 128:(two + 1) * 128].rearrange(
                            "c h w -> c (h w)"))

            def emit_binarize(n):
                # Writes the interior of all kw-copies. ALL on DVE: GPSIMD
                # tensor_scalar is several times slower and having matmul-
                # gating copies on it serialized the whole pipeline behind
                # GPSIMD (measured ~600us/rep). Conv drains live on ACT so
                # DVE's queue is binarize + weight drains only.
                for two in range(CI_CHUNKS):
                    xr_in = xraw_tiles[n][two][:].rearrange(
                        "c (h w) -> c h w", w=W)
                    for kw in range(NCOPY):
                        xg_in = xg5[:, kw, n, two,
                                    LEAD + DELTA[kw]:LEAD + DELTA[kw] + GRID
                                    ].rearrange("c (h w) -> c h w", w=WROW)
                        nc.vector.tensor_scalar(
                            xg_in[:, 1:H + 1, 1:W + 1],
                            xr_in[:],
                            0.0, 0.5, op0=ALU.is_ge, op1=ALU.subtract)

            # HAM warmup: the PE is idle until the first weight DMA lands
            # (~4us) and would then run its first ~3.4us of real work at
            # 1.2 GHz (cold clock-gate). Dummy identity matmuls during the
            # DMA wait release the throttle before the transposes start.
            # (Transpose-mode matmuls don't count as PE-busy for HAM.)
            if rep == 0:
                with tc.tile_pool(name="warm", bufs=1, space="PSUM") as wp:
                    warm = wp.tile([128, 128], F32, tag="warm", name="warm")
                    for i in range(8):
                        nc.tensor.matmul(warm[:], ident[:], ident[:],
                                         start=True, stop=True)

            # emission order = scheduling priority: cc0 weights, then the
            # x DMAs (SP ring starts draining immediately), image-0
            # binarize on GPSIMD, cc1 weights; images 1..3 binarize is
            # emitted interleaved with the conv groups below so the DVE
            # FIFO never parks a conv drain behind a late image's binarize
            n_inputs = NPC if parts not in ("mmonly", "mmraw", "mmraweven", "mmrawsame") else 0
            with tc.tile_pool(name="wtp", bufs=2, space="PSUM") as wtpsum:
                emit_weight_cc(0, wtpsum)
                if rep == 0 or n_inputs == 0:
                    # borders are static zeros: binarize only rewrites the
                    # interiors, so later reps reuse rep 0's borders (input-
                    # less ablations re-memset so xpall has a writer per rep)
                    emit_memsets()
                for n in range(n_inputs):
                    emit_xdma(n)
                if n_inputs:
                    emit_binarize(0)
                emit_weight_cc(1, wtpsum)
                if n_inputs > 1:
                    emit_binarize(1)
            # per image view [k, kw, two, s]: the two-axis (stride CHUNK,
            # %16==0) is the DoubleRow pair axis of the rhs
            xp = [xg5[:, :, n] for n in range(NPC)]

            # ---- conv phase ----
            # per-row-group PSUM tiles rotating through all 8 banks: group
            # g+1's first matmul into a bank only waits for a drain from
            # ~1.5 groups earlier, so TensorE never stalls on drains
            with tc.tile_pool(name="cpsum", bufs=8, space="PSUM") as cpsum:
                ngroups = NPC * CO_CHUNKS if parts != "nomm" else 0
                for gi in range(ngroups):
                    n, cc = divmod(gi, CO_CHUNKS)
                    if cc == 0 and 2 <= n + 1 < n_inputs:
                        emit_binarize(n + 1)
                    pps = [cpsum.tile([128, 512], F32, tag="cps",
                                      name=f"cps{rep}_{cc}_{n}_{rg}")
                           for rg in range(NROW_GROUPS)]
                    # rg-outer / tap-inner: consecutive matmuls stream
                    # 98%-overlapping rhs spans (offsets +-1, +-58), hitting
                    # the HW span-reuse fast path (measured: re-streaming an
                    # identical span is ~1.4x faster than a fresh one; LDW
                    # is emitted per-matmul either way, so tap-inner loses
                    # nothing on the weight side)
                    if parts == "wre":
                        # A/B probe: tap-outer / rg-inner — 8 consecutive
                        # matmuls share one lhsT; wins iff the backend
                        # dedups/hides consecutive identical weight loads
                        for ki, kpos in enumerate(range(KS * KS)):
                            kh, kw = divmod(kpos, KS)
                            lhsT = wd8[cc][:, kpos * 256:(kpos + 1) * 256] \
                                .rearrange("k (two m) -> k two m", two=2)
                            cp = kw if NCOPY == KS else 0
                            for rg in range(NROW_GROUPS):
                                off = (LEAD + DELTA[cp] + WROW + rg * FREE
                                       + (kh - 1) * WROW + (kw - 1))
                                rhs = xp[n][:, cp, :, off:off + FREE]
                                nc.tensor.matmul(
                                    pps[rg][:, :FREE], lhsT,
                                    rhs, start=(ki == 0),
                                    stop=(ki == KS * KS - 1),
                                    perf_mode=DR)
                    rgs = () if parts == "wre" else range(NROW_GROUPS)
                    for rg in rgs:
                        # tap order kh-major (kpos): measured equal-or-better
                        # than kw-major copy-grouped order, within noise
                        for ki, kpos in enumerate(range(KS * KS)):
                            kh, kw = divmod(kpos, KS)
                            lhsT = wd8[cc][:, kpos * 256:(kpos + 1) * 256] \
                                .rearrange("k (two m) -> k two m", two=2)
                            # span base inside the kw-copy: 16B-aligned by
                            # construction (LEAD+DELTA[kw]+(kw-1) == 64,
                            # and WROW, FREE are multiples of 16)
                            cp = kw if NCOPY == KS else 0
                            off = (LEAD + DELTA[cp] + WROW + rg * FREE
                                   + (kh - 1) * WROW + (kw - 1))
                            rhs = xp[n][:, cp, :, off:off + FREE]
                            nc.tensor.matmul(
                                pps[rg][:, :FREE], lhsT,
                                rhs, start=(ki == 0),
                                stop=(ki == KS * KS - 1),
                                perf_mode=DR)
                    ob = out_pool.tile([128, NROW_GROUPS * ROWS_PER_GROUP * W],
                                       F32, tag="ob",
                                       name=f"ob{rep}_{cc}_{n}")
                    # per-row-group drains (x4 restores the +-0.25 products),
                    # alternating ACT/DVE so the serial drain tail halves
                    # (mmraw/nodrain ablations: 8-col token drains that still
                    # read every PSUM bank so DCE keeps the matmuls)
                    ncol = W if parts not in ("mmraw", "mmraweven", "mmrawsame", "nodrain", "wreuseraw") else 8
                    for rg in range(NROW_GROUPS):
                        drain_in = pps[rg][:, :FREE] \
                            .rearrange("m (r c) -> m r c", c=WROW
                                       )[:, :, 1:ncol + 1]
                        drain_out = ob[:].rearrange(
                            "m (g r c) -> m g r c", g=NROW_GROUPS, c=W
                            )[:, rg, :, :ncol]
                        # drains on ACT; "ds" A/B probe alternates ACT/DVE
                        # so PSUM banks free twice as fast at group
                        # boundaries (zero spare banks with 8 groups)
                        if parts == "ds" and rg % 2 == 1:
                            nc.vector.tensor_scalar(
                                drain_out, drain_in,
                                4.0, bias_sb[:, cc:cc + 1],
                                op0=ALU.mult, op1=ALU.add)
                        else:
                            nc.scalar.activation(
                                drain_out, drain_in,
                                AF.Identity, bias=bias_sb[:, cc:cc + 1],
                                scale=4.0)
                    # x rides SWDGE now, so outputs take the two idle HWDGE
                    # rings (alternating SP/ACT); the last group is split so
                    # its early quarters overlap the final drains
                    ob_g = ob[:].rearrange("m (g s) -> m g s", g=NROW_GROUPS)
                    od_g = o_d3[n][cc].rearrange("c (g s) -> c g s",
                                                 g=NROW_GROUPS)
                    if parts in ("noout", "mmraw", "mmraweven", "mmrawsame", "nodrain", "wreuseraw"):
                        # tiny consumer keeps drains/MMs live through DCE
                        nc.gpsimd.dma_start(od_g[:, 0, :64], ob_g[:, 0, :64])
                    elif gi == ngroups - 1:
                        # both HWDGE rings are idle by now; alternate the
                        # quarters so the tail transfer time halves
                        for qi, (lo, hi) in enumerate(
                                ((0, 2), (2, 4), (4, 6), (6, NROW_GROUPS))):
                            eng = nc.sync if qi % 2 == 0 else nc.scalar
                            eng.dma_start(od_g[:, lo:hi], ob_g[:, lo:hi])
                    else:
                        eng = nc.sync if gi % 2 == 0 else nc.scalar
                        eng.dma_start(o_d3[n][cc], ob[:])


_nc_cache = {}


def _get_nc(repeats=1, parts="full"):
    key = (repeats, parts)
    if key not in _nc_cache:
        nc = bacc.Bacc("TRN2", debug=False)
        x_d = nc.dram_tensor("x", [NPC, CIN, H, W], F32, kind="ExternalInput").ap()
        w_d = nc.dram_tensor("w", [COUT, CIN, KS, KS], F32,
                             kind="ExternalInput").ap()
        b_d = nc.dram_tensor("b", [COUT], F32, kind="ExternalInput").ap()
        o_d = nc.dram_tensor("out", [NPC, COUT, H, W], F32,
                             kind="ExternalOutput").ap()
        with tile.TileContext(nc) as tc:
            _body(tc, x_d, w_d, b_d, o_d, repeats=repeats, parts=parts)
        nc.compile()
        # Guard: bass does NOT reliably assert SBUF capacity — an overrun
        # compiles fine, then wedges the device (observed: 237KB high-water
        # silently corrupted runtime state, NRT_EXEC_UNIT_UNRECOVERABLE).
        hi = 0
        for alloc in nc.m.functions[0].allocations:
            if isinstance(alloc, mybir.MemoryLocationSet) \
                    and alloc.kind == "Internal":
                for ml in alloc.memorylocations:
                    d = list(ml.dims)
                    if len(d) == 2 and d[0] == 128:
                        isz = np.dtype(mybir.dt.np(alloc.dtype)).itemsize \
                            if alloc.dtype else 1
                        hi = max(hi, ml.addr + d[1] * isz)
        assert hi <= 224 * 1024, f"SBUF high-water {hi} exceeds 224KiB"
        _nc_cache[key] = nc
    return _nc_cache[key]


# ---- persistent PJRT runner ---------------------------------------------
# bass_utils.run_bass_kernel_spmd builds a FRESH jax.jit closure per call, so
# every invocation re-lowers the module, re-ships the NEFF through axon, and
# re-loads it onto all 8 devices — seconds of overhead per call that has
# nothing to do with device execution. Here the jitted executable (and hence
# the loaded NEFF) is cached: the first call pays compile+load once, later
# calls only dispatch.

_runner_cache = {}


def _get_runner(repeats=1, parts="full"):
    key = (repeats, parts)
    if key in _runner_cache:
        return _runner_cache[key]

    import jax
    from jax.sharding import Mesh, PartitionSpec, NamedSharding
    from jax.experimental.shard_map import shard_map
    from concourse import bass2jax, mybir as mb

    nc = _get_nc(repeats, parts)
    bass2jax.install_neuronx_cc_hook()

    partition_name = (nc.partition_id_tensor.name
                      if nc.partition_id_tensor else None)
    in_names, out_names, out_avals, zero_shapes = [], [], [], []
    for alloc in nc.m.functions[0].allocations:
        if not isinstance(alloc, mb.MemoryLocationSet):
            continue
        name = alloc.memorylocations[0].name
        if alloc.kind == "ExternalInput":
            if name != partition_name:
                in_names.append(name)
        elif alloc.kind == "ExternalOutput":
            out_names.append(name)
            shape = tuple(alloc.tensor_shape)
            dtype = mb.dt.np(alloc.dtype)
            out_avals.append(jax.core.ShapedArray(shape, dtype))
            zero_shapes.append((shape, dtype))
    n_params = len(in_names)
    n_outs = len(out_names)
    in_names = in_names + out_names
    if partition_name is not None:
        in_names = in_names + [partition_name]

    def _body_fn(*args):
        operands = list(args)
        if partition_name is not None:
            operands.append(bass2jax.partition_id_tensor())
        outs = bass2jax._bass_exec_p.bind(
            *operands,
            out_avals=tuple(out_avals),
            in_names=tuple(in_names),
            out_names=tuple(out_names),
            lowering_input_output_aliases=(),
            sim_require_finite=True,
            sim_require_nnan=True,
            nc=nc,
        )
        return tuple(outs)

    devices = jax.devices()[:N_CORES]
    mesh = Mesh(np.asarray(devices), ("core",))
    sharding = NamedSharding(mesh, PartitionSpec("core"))
    donate = tuple(range(n_params, n_params + n_outs))
    fn = jax.jit(
        shard_map(
            _body_fn, mesh=mesh,
            in_specs=(PartitionSpec("core"),) * (n_params + n_outs),
            out_specs=(PartitionSpec("core"),) * n_outs,
            check_rep=False,
        ),
        donate_argnums=donate, keep_unused=True,
    )
    # On-device sharded zero buffers for the donated outputs (regenerated per
    # call — donation consumes them). No host->device traffic involved.
    import jax.numpy as jnp
    zeros_fn = jax.jit(
        lambda: tuple(jnp.zeros((N_CORES * s[0],) + s[1:], d)
                      for s, d in zero_shapes),
        out_shardings=(sharding,) * n_outs,
    )
    runner = {"fn": fn, "zeros_fn": zeros_fn, "sharding": sharding,
              "n_params": n_params, "in_order": in_names[:n_params]}
    _runner_cache[key] = runner
    return runner


def _device_inputs(inputs, runner):
    """Concat per-core shards on axis 0 and put on the 8 devices."""
    import jax
    x, w, b = inputs["x"], inputs["w"], inputs["b"]
    full = {
        "x": np.ascontiguousarray(x, dtype=np.float32),
        "w": np.ascontiguousarray(
            np.broadcast_to(w, (N_CORES,) + tuple(w.shape)).reshape(
                (N_CORES * w.shape[0],) + tuple(w.shape[1:])),
            dtype=np.float32),
        "b": np.ascontiguousarray(
            np.broadcast_to(b, (N_CORES,) + tuple(b.shape)).reshape(
                (N_CORES * b.shape[0],) + tuple(b.shape[1:])),
            dtype=np.float32),
    }
    arrs = [full[name] for name in runner["in_order"]]
    return jax.device_put(arrs, [runner["sharding"]] * len(arrs))


def _exec(runner, dev_in):
    outs = runner["fn"](*dev_in, *runner["zeros_fn"]())
    return outs


def _run(inputs, repeats=1, parts="full", **kwargs):
    x = inputs["x"]
    assert x.shape == (N, CIN, H, W), x.shape
    runner = _get_runner(repeats, parts)
    dev_in = _device_inputs(inputs, runner)
    outs = _exec(runner, dev_in)
    out = np.asarray(outs[0])

    class _Res:
        exec_time_ns = None
        instructions_and_trace = None
    return out, _Res()


def kernel(**inputs) -> np.ndarray:
    out, _ = _run(inputs)
    return out

